# revision 1
# baseline (speedup 1.0000x reference)
"""Causal self-attention (B=4, T=2048, D=1024, H=16) on 8 trn2 NeuronCores.

Sharding: core = b*2 + g  (b = batch 0..3, g = head-group 0..1, 8 heads each).
Each core computes, for its batch b and its 8 heads:
  qkv projection -> flash-style causal attention -> partial out-projection
  out_partial = att_out(b, heads_g) @ Wout[rows_g]        (2048, 1024) fp32
Host sums the two head-group partials per batch (the "all-reduce").

On-chip layout (all bf16 except PSUM/normalization):
  xT   [128, 8, 2048]  : x.T        (d-tile, t)      via PE transpose
  qT/kT[128, 4, 2048]  : q.T / k.T  head h -> tile h//2, partitions (h%2)*64+
  v    [128, 16, 8, 65]: v natural  (t-tile, head, dh | ones col for denom)
  oT   [128, 4, 2048]  : att_out.T  same head mapping as qT
Scores are computed transposed (S.T[tk, tq]), exp'ed with scale=1/8 on ACT,
causally masked via a 128x128 upper-tri tile, and contracted with V on the
partition (tk) axis; an appended ones-column in V yields the softmax
denominator for free in PSUM row 64.
"""
from contextlib import ExitStack

import numpy as np
import ml_dtypes

import concourse.bacc as bacc
import concourse.tile as tile
from concourse import bass_utils, mybir
from concourse.masks import make_identity

FP32 = mybir.dt.float32
BF16 = mybir.dt.bfloat16
EXP = mybir.ActivationFunctionType.Exp

B, T, D = 4, 2048, 1024
H_TOT, DH = 16, 64
NH = 8            # heads per core
NDT = 8           # d-tiles of 128 (D / 128)
NKT = 16          # t-tiles of 128
NTC = 4           # t-chunks of 512
CH = 512

_CACHE = {}


def _build():
    nc = bacc.Bacc("TRN2", target_bir_lowering=False, debug=False, num_devices=8)
    xb = nc.dram_tensor("xb", [T, D], BF16, kind="ExternalInput").ap()
    wqkv = nc.dram_tensor("wqkv", [D, 3 * CH], BF16, kind="ExternalInput").ap()
    wout = nc.dram_tensor("wout", [CH, D], BF16, kind="ExternalInput").ap()
    trid = nc.dram_tensor("tri", [128, 128], BF16, kind="ExternalInput").ap()
    outp = nc.dram_tensor("out_p", [T, D], FP32, kind="ExternalOutput").ap()

    with tile.TileContext(nc) as tc, ExitStack() as ctx:
        const = ctx.enter_context(tc.tile_pool(name="const", bufs=1))
        big = ctx.enter_context(tc.tile_pool(name="big", bufs=1))
        xload = ctx.enter_context(tc.tile_pool(name="xload", bufs=3))
        evs = ctx.enter_context(tc.tile_pool(name="evs", bufs=4))
        dn = ctx.enter_context(tc.tile_pool(name="dn", bufs=3))

        ident = const.tile([128, 128], BF16)
        make_identity(nc, ident)
        tri = const.tile([128, 128], BF16)
        nc.sync.dma_start(out=tri, in_=trid)

        wqkv_sb = big.tile([128, NDT, 3 * CH], BF16)
        nc.sync.dma_start(out=wqkv_sb, in_=wqkv.rearrange("(a p) c -> p a c", p=128))
        wout_sb = big.tile([128, NTC, D], BF16)
        nc.sync.dma_start(out=wout_sb, in_=wout.rearrange("(a p) c -> p a c", p=128))

        xT = big.tile([128, NDT, T], BF16)
        qT = big.tile([128, 4, T], BF16)
        kT = big.tile([128, 4, T], BF16)
        oT = big.tile([128, 4, T], BF16)
        v_sb = big.tile([128, NKT, NH, DH + 1], BF16)
        nc.vector.memset(v_sb[:, :, :, DH:DH + 1], 1.0)

        # ---- phase 0/1a: transpose x and project V ----
        with tc.tile_pool(name="ptr", bufs=2, space="PSUM") as ptr, \
             tc.tile_pool(name="pv", bufs=2, space="PSUM") as pv, \
             tc.tile_pool(name="pqk", bufs=2, space="PSUM") as pqk:
            for i in range(NKT):
                xt = xload.tile([128, D], BF16)
                nc.sync.dma_start(out=xt, in_=xb[i * 128:(i + 1) * 128, :])
                for d in range(NDT):
                    pt = ptr.tile([128, 128], BF16)
                    nc.tensor.transpose(pt, xt[:, d * 128:(d + 1) * 128], ident)
                    nc.vector.tensor_copy(out=xT[:, d, i * 128:(i + 1) * 128], in_=pt)
                pvt = pv.tile([128, CH], FP32)
                for d in range(NDT):
                    nc.tensor.matmul(pvt, xT[:, d, i * 128:(i + 1) * 128],
                                     wqkv_sb[:, d, 2 * CH:3 * CH],
                                     start=(d == 0), stop=(d == NDT - 1))
                nc.vector.tensor_copy(out=v_sb[:, i, :, 0:DH],
                                      in_=pvt.rearrange("p (h e) -> p h e", h=NH))

            # ---- phase 1b: project Q.T and K.T ----
            for ct in range(8):
                dst = qT if ct < 4 else kT
                pair = ct % 4
                for c in range(NTC):
                    pq = pqk.tile([128, CH], FP32)
                    for d in range(NDT):
                        nc.tensor.matmul(pq, wqkv_sb[:, d, ct * 128:(ct + 1) * 128],
                                         xT[:, d, c * CH:(c + 1) * CH],
                                         start=(d == 0), stop=(d == NDT - 1))
                    nc.vector.tensor_copy(out=dst[:, pair, c * CH:(c + 1) * CH], in_=pq)

        # ---- phase 2/3: causal flash attention, transposed scores ----
        with tc.tile_pool(name="pss", bufs=3, space="PSUM") as pss, \
             tc.tile_pool(name="po", bufs=2, space="PSUM") as po:
            for h in range(NH):
                pair, off = h // 2, (h % 2) * 64
                for c in range(NTC):
                    nkt = 4 * (c + 1)
                    pot = po.tile([DH + 1, CH], FP32)
                    ptiles = []
                    for kt in range(nkt):
                        d = kt - 4 * c       # >= 0: partial (diagonal) block
                        s = 128 * d if d > 0 else 0
                        psst = pss.tile([128, CH], FP32)
                        nc.tensor.matmul(psst[:, s:CH],
                                         kT[off:off + 64, pair, kt * 128:(kt + 1) * 128],
                                         qT[off:off + 64, pair, c * CH + s:(c + 1) * CH],
                                         start=True, stop=True)
                        ptile = evs.tile([128, CH], BF16)
                        if s > 0:
                            nc.vector.memset(ptile[:, 0:s], 0.0)
                        nc.scalar.activation(out=ptile[:, s:CH], in_=psst[:, s:CH],
                                             func=EXP, scale=0.125)
                        if d >= 0:
                            nc.vector.tensor_mul(ptile[:, s:s + 128],
                                                 ptile[:, s:s + 128], tri)
                        ptiles.append(ptile)
                        if kt >= 1:   # software-pipelined O matmul (prev block)
                            nc.tensor.matmul(pot, v_sb[:, kt - 1, h, :], ptiles[kt - 1],
                                             start=(kt == 1), stop=False)
                    nc.tensor.matmul(pot, v_sb[:, nkt - 1, h, :], ptiles[nkt - 1],
                                     start=(nkt == 1), stop=True)
                    den = dn.tile([1, CH], FP32)
                    nc.vector.reciprocal(den, pot[DH:DH + 1, :])
                    bc = dn.tile([64, CH], FP32)
                    nc.gpsimd.partition_broadcast(bc, den)
                    nc.vector.tensor_mul(oT[off:off + 64, pair, c * CH:(c + 1) * CH],
                                         pot[0:DH, :], bc)

        # ---- phase 4: partial out-projection ----
        with tc.tile_pool(name="pfin", bufs=4, space="PSUM") as pfin:
            for i in range(NKT):
                for n in range(2):
                    pf = pfin.tile([128, CH], FP32)
                    for dt in range(4):
                        nc.tensor.matmul(pf, oT[:, dt, i * 128:(i + 1) * 128],
                                         wout_sb[:, dt, n * CH:(n + 1) * CH],
                                         start=(dt == 0), stop=(dt == 3))
                    st = evs.tile([128, CH], FP32)
                    nc.vector.tensor_copy(out=st, in_=pf)
                    nc.sync.dma_start(
                        out=outp[i * 128:(i + 1) * 128, n * CH:(n + 1) * CH], in_=st)

    nc.compile()
    return nc


def _get_nc():
    if "nc" not in _CACHE:
        _CACHE["nc"] = _build()
    return _CACHE["nc"]


def kernel(x, causal_mask, Wqkv, Wout):
    nc = _get_nc()
    bf = ml_dtypes.bfloat16
    x_bf = np.ascontiguousarray(x).astype(bf)               # (4, 2048, 1024)
    tri = np.triu(np.ones((128, 128), np.float32)).astype(bf)
    wq_g, wo_g = [], []
    for g in range(2):
        sl = slice(g * CH, (g + 1) * CH)
        wq_g.append(np.ascontiguousarray(np.concatenate(
            [Wqkv[:, :D][:, sl], Wqkv[:, D:2 * D][:, sl], Wqkv[:, 2 * D:][:, sl]],
            axis=1)).astype(bf))
        wo_g.append(np.ascontiguousarray(Wout[sl, :]).astype(bf))

    in_maps = []
    for core in range(8):
        b, g = core // 2, core % 2
        in_maps.append({"xb": np.ascontiguousarray(x_bf[b]),
                        "wqkv": wq_g[g], "wout": wo_g[g], "tri": tri})
    res = bass_utils.run_bass_kernel_spmd(nc, in_maps, list(range(8)))
    out = np.empty((B, T, D), np.float32)
    for b in range(B):
        out[b] = res.results[2 * b]["out_p"] + res.results[2 * b + 1]["out_p"]
    return out
